# revision 7
# baseline (speedup 1.0000x reference)
"""Trainium2 Bass kernel for nn_DescriptorNetwork (gnn_message_passing).

Data-parallel over crystals: 8192 crystals sharded 1024-per-core across 8
NeuronCores; params replicated. Each crystal has E=4 nodes, fully connected
(16 edges). All segment reductions are device-local groups of 4.

Layout: feature-major activations xT [128 part, 2 k-tiles, 4096 nodes],
fp32r matmuls (full PE rate), pair-MLP decomposed into self/nbr halves with
broadcast-pattern tensor_tensor adds, attention gates computed via a
shifted-column stationary matrix so each 256-node chunk's gate logits land
on their own PSUM partition row.
"""

import contextlib

import numpy as np

import concourse.bass as bass
import concourse.mybir as mybir
import concourse.tile as tile
from concourse import bacc
from concourse.bass_utils import run_bass_kernel_spmd

AF = mybir.ActivationFunctionType
ALU = mybir.AluOpType
X_AX = mybir.AxisListType.X
XY_AX = mybir.AxisListType.XY
F32 = mybir.dt.float32
F32R = mybir.dt.float32r

# Problem constants (hardcoded per spec)
NCORES = 8
C_TOT, E = 8192, 4
N_TOT = C_TOT * E          # 32768
ELEM_EMB, FEA, HID = 200, 256, 512
N_GRAPH, HEADS = 3, 3
SLOPE = 0.01

# Per-core sizes
C_LOC = C_TOT // NCORES    # 1024 crystals
N_LOC = C_LOC * E          # 4096 nodes
M_LOC = N_LOC * E          # 16384 edges
NCH = 256                  # nodes per chunk
CHUNKS = N_LOC // NCH      # 16
ECH = NCH * E              # 1024 edges per chunk

# Shifted gate-weight buffer: w2 at col P, window c = [P-c, P-c+128)
WSH_P = CHUNKS - 1         # 15
WSH_W = WSH_P + 128        # 143

# Weight blob column offsets
WG_COLS = 2 * 512 * 2 + 4 * WSH_W            # 2620
WM_COLS = 2 * 512 * 2 + 4 * 256              # 3072
CW_COLS = 2 * 512 * 2 + 4 * WSH_W + 4 * 256  # 3644

_NC_CACHE = {}


def _edge_aps(a_base, b_base, nq):
    """Read-APs for the pair expansion over nq crystals (4*nq nodes).

    Edge order e = 16q + 4a + b (a=self offset, b=nbr offset).
    E_A[e] = A[4q+a]  -> [[1, 4nq], [0, 4]]   (each col repeated 4x)
    E_B[e] = B[4q+b]  -> [[4, nq], [0, 4], [1, 4]]
    """
    ea = bass.AP(tensor=a_base.tensor, offset=a_base.offset,
                 ap=[a_base.ap[0], [1, 4 * nq], [0, 4]])
    eb = bass.AP(tensor=b_base.tensor, offset=b_base.offset,
                 ap=[b_base.ap[0], [4, nq], [0, 4], [1, 4]])
    return ea, eb


def _bcast4(t, n):
    """[16, n] tile -> [16, n, 4] broadcast AP (each col repeated 4x)."""
    return bass.AP(tensor=t.tensor, offset=t.offset,
                   ap=[t.ap[0], [1, n], [0, 4]])


def build_nc(pow_g, pow_c):
    nc = bacc.Bacc("TRN2", target_bir_lowering=False)

    xfea = nc.dram_tensor("xfea", [256, N_LOC], F32R, kind="ExternalInput")
    ew = nc.dram_tensor("ew", [N_LOC], F32, kind="ExternalInput")
    ones = nc.dram_tensor("ones", [1, 512], F32R, kind="ExternalInput")
    embw = nc.dram_tensor("embw", [128, 2 * 256], F32R, kind="ExternalInput")
    embb = nc.dram_tensor("embb", [1, 256], F32R, kind="ExternalInput")
    hcb = nc.dram_tensor("hcb", [128, 8], F32, kind="ExternalInput")
    gw = [[nc.dram_tensor(f"g{L}h{h}w", [128, WG_COLS], F32R, kind="ExternalInput")
           for h in range(HEADS)] for L in range(N_GRAPH)]
    mw = [[nc.dram_tensor(f"g{L}h{h}m", [128, WM_COLS], F32R, kind="ExternalInput")
           for h in range(HEADS)] for L in range(N_GRAPH)]
    gb = [[nc.dram_tensor(f"g{L}h{h}b", [1, 1024], F32R, kind="ExternalInput")
           for h in range(HEADS)] for L in range(N_GRAPH)]
    cw = [nc.dram_tensor(f"c{h}w", [128, CW_COLS], F32R, kind="ExternalInput")
          for h in range(HEADS)]
    cb = [nc.dram_tensor(f"c{h}b", [1, 1024], F32R, kind="ExternalInput")
          for h in range(HEADS)]
    outT = nc.dram_tensor("outT", [256, C_LOC], F32, kind="ExternalOutput")

    with tile.TileContext(nc) as tc, contextlib.ExitStack() as ctx:
        consts = ctx.enter_context(tc.tile_pool(name="consts", bufs=1))
        state = ctx.enter_context(tc.tile_pool(name="state", bufs=1))

        ones_sb = consts.tile([1, 512], F32R)
        nc.sync.dma_start(out=ones_sb, in_=ones[:, :])
        hcb_sb = consts.tile([128, 8], F32)
        nc.sync.dma_start(out=hcb_sb, in_=hcb[:, :])
        lnw = consts.tile([16, 256], F32)
        ew_sb = consts.tile([16, 256], F32)
        nc.sync.dma_start(out=ew_sb, in_=ew.ap().rearrange("(p n) -> p n", p=16))
        nc.scalar.activation(out=lnw, in_=ew_sb, func=AF.Ln)

        xT = state.tile([128, 2, N_LOC], F32R)

        # ---------------- embed ----------------
        with tc.tile_pool(name="xfeap", bufs=1) as xfp, \
             tc.tile_pool(name="embps", bufs=2, space="PSUM") as embps:
            xf = xfp.tile([128, 2, N_LOC], F32R)
            nc.sync.dma_start(out=xf, in_=xfea.ap().rearrange("(k p) n -> p k n", p=128))
            ew_t = xfp.tile([128, 2 * 256], F32R)
            nc.sync.dma_start(out=ew_t, in_=embw[:, :])
            eb_t = xfp.tile([1, 256], F32R)
            nc.sync.dma_start(out=eb_t, in_=embb[:, :])
            for c in range(8):
                cs = slice(512 * c, 512 * (c + 1))
                xp = embps.tile([128, 2, 512], F32)
                for mt in range(2):
                    for kt in range(2):
                        nc.tensor.matmul(
                            xp[:, mt, :],
                            ew_t[:, kt * 256 + mt * 128: kt * 256 + (mt + 1) * 128],
                            xf[:, kt, cs], start=(kt == 0), stop=False)
                    nc.tensor.matmul(
                        xp[:, mt, :], eb_t[0:1, mt * 128:(mt + 1) * 128],
                        ones_sb[0:1, 0:512], start=False, stop=True)
                nc.scalar.copy(out=xT[:, :, cs], in_=xp)

        # ---------------- graph layers ----------------
        with tc.tile_pool(name="wgp", bufs=2) as wgp, \
             tc.tile_pool(name="wmp", bufs=3) as wmp, \
             tc.tile_pool(name="wbp", bufs=3) as wbp, \
             tc.tile_pool(name="uhp", bufs=1) as uhp, \
             tc.tile_pool(name="drp", bufs=1) as drp, \
             tc.tile_pool(name="grp", bufs=2) as grp, \
             tc.tile_pool(name="smx", bufs=1) as smx, \
             tc.tile_pool(name="gfp", bufs=3) as gfp, \
             tc.tile_pool(name="gmp", bufs=1) as gmp, \
             tc.tile_pool(name="rdp", bufs=2) as rdp:
            for L in range(N_GRAPH):
                wg_t = [wgp.tile([128, WG_COLS], F32R, tag="wg", name=f"wgt{L}_{i}") for i in range(HEADS)]
                wm_t = [wmp.tile([128, WM_COLS], F32R, tag="wm", name=f"wmt{L}_{i}") for i in range(HEADS)]
                gb_t = [wbp.tile([1, 1024], F32R, tag="wb", name=f"gbt{L}_{i}") for i in range(HEADS)]
                for h in range(HEADS):
                    nc.sync.dma_start(out=wm_t[h], in_=mw[L][h][:, :])
                    nc.sync.dma_start(out=gb_t[h], in_=gb[L][h][:, :])

                # ---- PASS G: gates per head ----
                gfin = []
                for h in range(HEADS):
                    nc.sync.dma_start(out=wg_t[h], in_=gw[L][h][:, :])
                    wgh, gbh = wg_t[h], gb_t[h]
                    wp = smx.tile([16, 256], F32, tag="wp")
                    nc.scalar.activation(out=wp, in_=lnw, func=AF.Exp,
                                         scale=float(pow_g[L][h]))
                    with tc.tile_pool(name="gpsG", bufs=1, space="PSUM") as gps, \
                         tc.tile_pool(name="abG", bufs=1, space="PSUM") as abps:
                        g_ps = gps.tile([128, 1024], F32, tag="gps")
                        for c in range(CHUNKS):
                            cs = slice(NCH * c, NCH * (c + 1))
                            ab = abps.tile([128, 8, 256], F32, tag="ab")
                            for mt in range(4):
                                for kt in range(2):
                                    nc.tensor.matmul(
                                        ab[:, mt, :],
                                        wgh[:, kt * 512 + mt * 128: kt * 512 + (mt + 1) * 128],
                                        xT[:, kt, cs], start=(kt == 0), stop=False)
                                nc.tensor.matmul(
                                    ab[:, mt, :], gbh[0:1, mt * 128:(mt + 1) * 128],
                                    ones_sb[0:1, 0:256], start=False, stop=True)
                                for kt in range(2):
                                    nc.tensor.matmul(
                                        ab[:, 4 + mt, :],
                                        wgh[:, 1024 + kt * 512 + mt * 128:
                                            1024 + kt * 512 + (mt + 1) * 128],
                                        xT[:, kt, cs], start=(kt == 0), stop=(kt == 1))
                            bsb = drp.tile([128, 4, 256], F32, tag="dr1")
                            nc.scalar.copy(out=bsb, in_=ab[:, 4:8, :])
                            ug = uhp.tile([128, 4, 1024], F32, tag="u")
                            for mt in range(4):
                                ea, ebp = _edge_aps(ab[:, mt, :], bsb[:, mt, :], 64)
                                nc.vector.tensor_tensor(out=ug[:, mt, :], in0=ebp,
                                                        in1=ea, op=ALU.add)
                            hg = uhp.tile([128, 4, 1024], F32R, tag="h")
                            nc.scalar.activation(out=hg, in_=ug, func=AF.Lrelu,
                                                 alpha=SLOPE)
                            for kt in range(4):
                                base = 2048 + kt * WSH_W + WSH_P - c
                                for hf in range(2):
                                    nc.tensor.matmul(
                                        g_ps[:, hf * 512:(hf + 1) * 512],
                                        wgh[:, base: base + 128],
                                        hg[:, kt, hf * 512:(hf + 1) * 512],
                                        start=(c == 0 and kt == 0),
                                        stop=(c == CHUNKS - 1 and kt == 3),
                                        skip_group_check=True)
                        gates = smx.tile([16, 1024], F32, tag="gates")
                        nc.vector.tensor_copy(out=gates, in_=g_ps[0:16, :])
                    # softmax over groups of 4 (in-place on `gates`)
                    gmax = smx.tile([16, 256], F32, tag="gmax")
                    nc.vector.tensor_reduce(
                        out=gmax, in_=gates.rearrange("p (g j) -> p g j", j=4),
                        axis=X_AX, op=ALU.max)
                    nc.vector.tensor_tensor(out=gates, in0=gates,
                                            in1=_bcast4(gmax, 256), op=ALU.subtract)
                    nc.scalar.activation(out=gates, in_=gates, func=AF.Exp)
                    wpe = bass.AP(tensor=wp.tensor, offset=wp.offset,
                                  ap=[wp.ap[0], [4, 64], [0, 4], [1, 4]])
                    gv = gates.rearrange("p (a b c) -> p a b c", b=4, c=4)
                    nc.vector.tensor_tensor(out=gv, in0=wpe, in1=gv, op=ALU.mult)
                    ssum = smx.tile([16, 256], F32, tag="ssum")
                    nc.vector.tensor_reduce(
                        out=ssum, in_=gates.rearrange("p (g j) -> p g j", j=4),
                        axis=X_AX, op=ALU.add)
                    nc.vector.tensor_scalar_add(out=ssum, in0=ssum, scalar1=1e-10)
                    rinv = smx.tile([16, 256], F32, tag="rinv")
                    nc.vector.reciprocal(out=rinv, in_=ssum)
                    gf = gfp.tile([16, 1024], F32, tag="gf")
                    nc.vector.tensor_tensor(out=gf, in0=gates,
                                            in1=_bcast4(rinv, 256), op=ALU.mult)
                    gfin.append(gf)

                # ---- PASS M: messages, chunks outer / heads inner ----
                with tc.tile_pool(name="mpsM", bufs=1, space="PSUM") as mps, \
                     tc.tile_pool(name="abM", bufs=1, space="PSUM") as abps:
                    for c in range(CHUNKS):
                        cs = slice(NCH * c, NCH * (c + 1))
                        gmsg = gmp.tile([128, 2, 3, 1024], F32, tag="gmsg")
                        for h in range(HEADS):
                            wmh, gbh = wm_t[h], gb_t[h]
                            ab = abps.tile([128, 8, 256], F32, tag="ab")
                            for mt in range(4):
                                for kt in range(2):
                                    nc.tensor.matmul(
                                        ab[:, mt, :],
                                        wmh[:, kt * 512 + mt * 128: kt * 512 + (mt + 1) * 128],
                                        xT[:, kt, cs], start=(kt == 0), stop=False)
                                nc.tensor.matmul(
                                    ab[:, mt, :],
                                    gbh[0:1, 512 + mt * 128: 512 + (mt + 1) * 128],
                                    ones_sb[0:1, 0:256], start=False, stop=True)
                                for kt in range(2):
                                    nc.tensor.matmul(
                                        ab[:, 4 + mt, :],
                                        wmh[:, 1024 + kt * 512 + mt * 128:
                                            1024 + kt * 512 + (mt + 1) * 128],
                                        xT[:, kt, cs], start=(kt == 0), stop=(kt == 1))
                            asb = drp.tile([128, 4, 256], F32, tag="dr1")
                            nc.scalar.copy(out=asb, in_=ab[:, 0:4, :])
                            bsb = drp.tile([128, 4, 256], F32, tag="dr2")
                            nc.vector.tensor_copy(out=bsb, in_=ab[:, 4:8, :])
                            um = uhp.tile([128, 4, 1024], F32, tag="u")
                            for mt in range(4):
                                ea, ebp = _edge_aps(asb[:, mt, :], bsb[:, mt, :], 64)
                                nc.gpsimd.tensor_tensor(out=um[:, mt, :], in0=ebp,
                                                        in1=ea, op=ALU.add)
                            hm = uhp.tile([128, 4, 1024], F32R, tag="h")
                            nc.scalar.activation(out=hm, in_=um, func=AF.Lrelu,
                                                 alpha=SLOPE)
                            mp = mps.tile([128, 2, 1024], F32, tag="mp")
                            for ft in range(2):
                                for kt in range(4):
                                    for hf in range(2):
                                        nc.tensor.matmul(
                                            mp[:, ft, hf * 512:(hf + 1) * 512],
                                            wmh[:, 2048 + kt * 256 + ft * 128:
                                                2048 + kt * 256 + (ft + 1) * 128],
                                            hm[:, kt, hf * 512:(hf + 1) * 512],
                                            start=(kt == 0), stop=(kt == 3),
                                            skip_group_check=True)
                            gsrc = grp.tile([1, 1024], F32, tag="gsrc")
                            nc.sync.dma_start(out=gsrc, in_=gfin[h][c:c + 1, :])
                            grep = grp.tile([128, 1024], F32, tag="grep")
                            nc.gpsimd.partition_broadcast(grep, gsrc[0:1, :])
                            for ft in range(2):
                                nc.vector.tensor_tensor(out=gmsg[:, ft, h, :],
                                                        in0=mp[:, ft, :], in1=grep,
                                                        op=ALU.mult)
                        for ft in range(2):
                            red = rdp.tile([128, 256], F32, tag="red")
                            b0 = gmsg[:, ft, 0, :]
                            rin = bass.AP(tensor=b0.tensor, offset=b0.offset,
                                          ap=[b0.ap[0], [4, 256], [1024, 3], [1, 4]])
                            nc.vector.tensor_reduce(out=red, in_=rin, axis=XY_AX,
                                                    op=ALU.add)
                            hb = hcb_sb[:, 2 * L + ft: 2 * L + ft + 1]
                            hbb = bass.AP(tensor=hb.tensor, offset=hb.offset,
                                          ap=[hb.ap[0], [0, 256]])
                            nc.gpsimd.tensor_tensor(out=red, in0=red, in1=hbb,
                                                    op=ALU.add)
                            nc.gpsimd.tensor_tensor(out=xT[:, ft, cs], in0=red,
                                                    in1=xT[:, ft, cs], op=ALU.add)

        # ---------------- crystal pooling ----------------
        with tc.tile_pool(name="cwp", bufs=3) as cwp, \
             tc.tile_pool(name="cstate", bufs=1) as cstate, \
             tc.tile_pool(name="cuh", bufs=2) as cuh, \
             tc.tile_pool(name="cgr", bufs=1) as cgr, \
             tc.tile_pool(name="csx", bufs=1) as csx, \
             tc.tile_pool(name="cgf", bufs=3) as cgfp, \
             tc.tile_pool(name="cgm", bufs=1) as cgm:
            ocry = cstate.tile([128, 2, C_LOC], F32)
            cw_t = [cwp.tile([128, CW_COLS], F32R, tag="cw", name=f"cwt{i}") for i in range(HEADS)]
            cb_t = [cwp.tile([1, 1024], F32R, tag="cb", name=f"cbt{i}") for i in range(HEADS)]
            for h in range(HEADS):
                nc.sync.dma_start(out=cw_t[h], in_=cw[h][:, :])
                nc.sync.dma_start(out=cb_t[h], in_=cb[h][:, :])
            cfin = []
            for h in range(HEADS):
                wch, cbh = cw_t[h], cb_t[h]
                wp = csx.tile([16, 256], F32, tag="cwpow")
                nc.scalar.activation(out=wp, in_=lnw, func=AF.Exp,
                                     scale=float(pow_c[h]))
                with tc.tile_pool(name="cgpsG", bufs=1, space="PSUM") as gps, \
                     tc.tile_pool(name="cabG", bufs=1, space="PSUM") as abps:
                    g_ps = gps.tile([128, 256], F32, tag="cgps")
                    for c in range(CHUNKS):
                        cs = slice(NCH * c, NCH * (c + 1))
                        ap_ = abps.tile([128, 4, 256], F32, tag="cab")
                        for mt in range(4):
                            for kt in range(2):
                                nc.tensor.matmul(
                                    ap_[:, mt, :],
                                    wch[:, kt * 512 + mt * 128: kt * 512 + (mt + 1) * 128],
                                    xT[:, kt, cs], start=(kt == 0), stop=False)
                            nc.tensor.matmul(
                                ap_[:, mt, :], cbh[0:1, mt * 128:(mt + 1) * 128],
                                ones_sb[0:1, 0:256], start=False, stop=True)
                        hcr = cuh.tile([128, 4, 256], F32R, tag="cuh")
                        nc.scalar.activation(out=hcr, in_=ap_, func=AF.Lrelu,
                                             alpha=SLOPE)
                        for kt in range(4):
                            base = 2048 + kt * WSH_W + WSH_P - c
                            nc.tensor.matmul(
                                g_ps[:, :], wch[:, base: base + 128], hcr[:, kt, :],
                                start=(c == 0 and kt == 0),
                                stop=(c == CHUNKS - 1 and kt == 3),
                                skip_group_check=True)
                    gates = csx.tile([16, 256], F32, tag="cgates")
                    nc.vector.tensor_copy(out=gates, in_=g_ps[0:16, :])
                gmax = csx.tile([16, 64], F32, tag="cgmax")
                nc.vector.tensor_reduce(
                    out=gmax, in_=gates.rearrange("p (g j) -> p g j", j=4),
                    axis=X_AX, op=ALU.max)
                nc.vector.tensor_tensor(out=gates, in0=gates,
                                        in1=_bcast4(gmax, 64), op=ALU.subtract)
                nc.scalar.activation(out=gates, in_=gates, func=AF.Exp)
                nc.vector.tensor_tensor(out=gates, in0=gates, in1=wp, op=ALU.mult)
                ssum = csx.tile([16, 64], F32, tag="cssum")
                nc.vector.tensor_reduce(
                    out=ssum, in_=gates.rearrange("p (g j) -> p g j", j=4),
                    axis=X_AX, op=ALU.add)
                nc.vector.tensor_scalar_add(out=ssum, in0=ssum, scalar1=1e-10)
                rinv = csx.tile([16, 64], F32, tag="crinv")
                nc.vector.reciprocal(out=rinv, in_=ssum)
                gf = cgfp.tile([16, 256], F32, tag="cgf")
                nc.vector.tensor_tensor(out=gf, in0=gates, in1=_bcast4(rinv, 64),
                                        op=ALU.mult)
                cfin.append(gf)

            with tc.tile_pool(name="cmpsM", bufs=1, space="PSUM") as mps, \
                 tc.tile_pool(name="cabM", bufs=1, space="PSUM") as abps:
                for c in range(CHUNKS):
                    cs = slice(NCH * c, NCH * (c + 1))
                    gmsg = cgm.tile([128, 2, 3, 256], F32, tag="cgmsg")
                    for h in range(HEADS):
                        wch, cbh = cw_t[h], cb_t[h]
                        ap_ = abps.tile([128, 4, 256], F32, tag="cab")
                        for mt in range(4):
                            for kt in range(2):
                                nc.tensor.matmul(
                                    ap_[:, mt, :],
                                    wch[:, 1024 + kt * 512 + mt * 128:
                                        1024 + kt * 512 + (mt + 1) * 128],
                                    xT[:, kt, cs], start=(kt == 0), stop=False)
                            nc.tensor.matmul(
                                ap_[:, mt, :],
                                cbh[0:1, 512 + mt * 128: 512 + (mt + 1) * 128],
                                ones_sb[0:1, 0:256], start=False, stop=True)
                        hcr = cuh.tile([128, 4, 256], F32R, tag="cuh")
                        nc.scalar.activation(out=hcr, in_=ap_, func=AF.Lrelu,
                                             alpha=SLOPE)
                        mp = mps.tile([128, 2, 256], F32, tag="cmp")
                        for ft in range(2):
                            for kt in range(4):
                                nc.tensor.matmul(
                                    mp[:, ft, :],
                                    wch[:, 2620 + kt * 256 + ft * 128:
                                        2620 + kt * 256 + (ft + 1) * 128],
                                    hcr[:, kt, :], start=(kt == 0), stop=(kt == 3),
                                    skip_group_check=True)
                        csrc = cgr.tile([1, 256], F32, tag="csrc")
                        nc.sync.dma_start(out=csrc, in_=cfin[h][c:c + 1, :])
                        grep = cgr.tile([128, 256], F32, tag="cgrep")
                        nc.gpsimd.partition_broadcast(grep, csrc[0:1, :])
                        for ft in range(2):
                            nc.vector.tensor_tensor(out=gmsg[:, ft, h, :],
                                                    in0=mp[:, ft, :], in1=grep,
                                                    op=ALU.mult)
                    for ft in range(2):
                        b0 = gmsg[:, ft, 0, :]
                        rin = bass.AP(tensor=b0.tensor, offset=b0.offset,
                                      ap=[b0.ap[0], [4, 64], [256, 3], [1, 4]])
                        nc.vector.tensor_reduce(out=ocry[:, ft, 64 * c: 64 * (c + 1)],
                                                in_=rin, axis=XY_AX, op=ALU.add)
            for ft in range(2):
                nc.vector.tensor_scalar_add(out=ocry[:, ft, :], in0=ocry[:, ft, :],
                                            scalar1=hcb_sb[:, 6 + ft: 7 + ft])
            nc.sync.dma_start(out=outT.ap().rearrange("(k p) n -> p k n", p=128),
                              in_=ocry)

    nc.finalize()
    return nc


def _prep_weights(params):
    """Pack parameter pytree into the DRAM blob arrays (host-side, f32)."""
    g = lambda x: np.asarray(x, dtype=np.float32)
    blobs = {}

    def pack_fc1(W):
        din = W.shape[0]
        nk = din // 128
        out = np.zeros((128, nk * 512), np.float32)
        for kt in range(nk):
            out[:, kt * 512:(kt + 1) * 512] = W[kt * 128:(kt + 1) * 128, :]
        return out

    def pack_w2sh(w2):
        out = np.zeros((128, 4 * WSH_W), np.float32)
        for kt in range(4):
            out[:, kt * WSH_W + WSH_P] = w2[kt * 128:(kt + 1) * 128, 0]
        return out

    def pack_w2m(W2):
        out = np.zeros((128, 4 * 256), np.float32)
        for kt in range(4):
            out[:, kt * 256:(kt + 1) * 256] = W2[kt * 128:(kt + 1) * 128, :] / 3.0
        return out

    pow_g = [[0.0] * HEADS for _ in range(N_GRAPH)]
    pow_c = [0.0] * HEADS
    for L in range(N_GRAPH):
        for h in range(HEADS):
            p = params["graphs"][L][h]
            Wg, Wm = g(p["gate"]["fc1"]["W"]), g(p["msg"]["fc1"]["W"])
            blobs[f"g{L}h{h}w"] = np.concatenate(
                [pack_fc1(Wg[:256]), pack_fc1(Wg[256:]),
                 pack_w2sh(g(p["gate"]["out"]["W"]))], axis=1)
            blobs[f"g{L}h{h}m"] = np.concatenate(
                [pack_fc1(Wm[:256]), pack_fc1(Wm[256:]),
                 pack_w2m(g(p["msg"]["out"]["W"]))], axis=1)
            barr = np.zeros((1, 1024), np.float32)
            barr[0, :512] = g(p["gate"]["fc1"]["b"])
            barr[0, 512:] = g(p["msg"]["fc1"]["b"])
            blobs[f"g{L}h{h}b"] = barr
            pow_g[L][h] = float(np.asarray(p["pow"]).reshape(-1)[0])
    for h in range(HEADS):
        p = params["cry_pool"][h]
        blobs[f"c{h}w"] = np.concatenate(
            [pack_fc1(g(p["gate"]["fc1"]["W"])), pack_fc1(g(p["msg"]["fc1"]["W"])),
             pack_w2sh(g(p["gate"]["out"]["W"])),
             pack_w2m(g(p["msg"]["out"]["W"]))], axis=1)
        barr = np.zeros((1, 1024), np.float32)
        barr[0, :512] = g(p["gate"]["fc1"]["b"])
        barr[0, 512:] = g(p["msg"]["fc1"]["b"])
        blobs[f"c{h}b"] = barr
        pow_c[h] = float(np.asarray(p["pow"]).reshape(-1)[0])

    embW = g(params["embed"]["W"])
    Wt = np.zeros((256, 256), np.float32)
    Wt[:200, :255] = embW
    Wt[200, 255] = 1.0
    embarr = np.zeros((128, 512), np.float32)
    for kt in range(2):
        embarr[:, kt * 256:(kt + 1) * 256] = Wt[kt * 128:(kt + 1) * 128, :]
    blobs["embw"] = embarr
    ebarr = np.zeros((1, 256), np.float32)
    ebarr[0, :255] = g(params["embed"]["b"])
    blobs["embb"] = ebarr

    hcbarr = np.zeros((128, 8), np.float32)
    for L in range(N_GRAPH):
        b = sum(g(params["graphs"][L][h]["msg"]["out"]["b"]) for h in range(HEADS)) / 3.0
        hcbarr[:, 2 * L] = b[:128]
        hcbarr[:, 2 * L + 1] = b[128:]
    bc = sum(g(params["cry_pool"][h]["msg"]["out"]["b"]) for h in range(HEADS)) / 3.0
    hcbarr[:, 6] = bc[:128]
    hcbarr[:, 7] = bc[128:]
    blobs["hcb"] = hcbarr
    blobs["ones"] = np.ones((1, 512), np.float32)
    return blobs, pow_g, pow_c


def _check_indices(self_idx, nbr_idx, cry_idx):
    node = np.arange(N_TOT, dtype=np.int64).reshape(C_TOT, E)
    exp_self = np.repeat(node, E, axis=1).reshape(-1)
    exp_nbr = np.tile(node, (1, E)).reshape(-1)
    exp_cry = np.repeat(np.arange(C_TOT, dtype=np.int64), E)
    return (np.array_equal(np.asarray(self_idx, np.int64), exp_self)
            and np.array_equal(np.asarray(nbr_idx, np.int64), exp_nbr)
            and np.array_equal(np.asarray(cry_idx, np.int64), exp_cry))


def _numpy_reference(elem_weights, elem_fea, self_idx, nbr_idx, cry_idx, params):
    """Pure-numpy fallback used only if the index structure ever differs."""
    g = lambda x: np.asarray(x, dtype=np.float32)

    def net(p, x):
        h = x @ g(p["fc1"]["W"]) + g(p["fc1"]["b"])
        h = np.where(h > 0, h, SLOPE * h)
        return h @ g(p["out"]["W"]) + g(p["out"]["b"])

    def seg_max(x, idx, n):
        out = np.full((n,) + x.shape[1:], -np.inf, np.float32)
        np.maximum.at(out, idx, x)
        return out

    def seg_sum(x, idx, n):
        out = np.zeros((n,) + x.shape[1:], np.float32)
        np.add.at(out, idx, x)
        return out

    def attn_pool(p, x, idx, w, n):
        gate = net(p["gate"], x)
        gate = gate - seg_max(gate, idx, n)[idx]
        gate = (w ** float(np.asarray(p["pow"]).reshape(-1)[0])) * np.exp(gate)
        gate = gate / (seg_sum(gate, idx, n)[idx] + 1e-10)
        return seg_sum(gate * net(p["msg"], x), idx, n)

    ew = g(elem_weights)
    x = g(elem_fea) @ g(params["embed"]["W"]) + g(params["embed"]["b"])
    x = np.concatenate([x, ew], axis=1)
    for heads in params["graphs"]:
        pair = np.concatenate([x[self_idx], x[nbr_idx]], axis=1)
        nw = ew[nbr_idx]
        msg = np.mean([attn_pool(p, pair, self_idx, nw, N_TOT) for p in heads], axis=0)
        x = msg + x
    return np.mean([attn_pool(p, x, cry_idx, ew, C_TOT) for p in params["cry_pool"]],
                   axis=0)


def kernel(elem_weights, elem_fea, self_fea_idx, nbr_fea_idx, cry_elem_idx, params):
    elem_weights = np.asarray(elem_weights, dtype=np.float32)
    elem_fea = np.asarray(elem_fea, dtype=np.float32)
    if not _check_indices(self_fea_idx, nbr_fea_idx, cry_elem_idx):
        return _numpy_reference(elem_weights, elem_fea,
                                np.asarray(self_fea_idx, np.int64),
                                np.asarray(nbr_fea_idx, np.int64),
                                np.asarray(cry_elem_idx, np.int64), params)

    blobs, pow_g, pow_c = _prep_weights(params)
    key = (tuple(tuple(r) for r in pow_g), tuple(pow_c))
    if key not in _NC_CACHE:
        _NC_CACHE[key] = build_nc(pow_g, pow_c)
    nc = _NC_CACHE[key]

    in_maps = []
    for k in range(NCORES):
        ns = slice(N_LOC * k, N_LOC * (k + 1))
        xfea = np.zeros((256, N_LOC), np.float32)
        xfea[:200, :] = elem_fea[ns].T
        xfea[200, :] = elem_weights[ns, 0]
        m = dict(blobs)
        m["xfea"] = xfea
        m["ew"] = np.ascontiguousarray(elem_weights[ns, 0])
        in_maps.append(m)

    res = run_bass_kernel_spmd(nc, in_maps, core_ids=list(range(NCORES)))
    out = np.empty((C_TOT, FEA), np.float32)
    for k in range(NCORES):
        out[C_LOC * k: C_LOC * (k + 1), :] = res.results[k]["outT"].T
    return out


def timed_run(np_inputs):
    """Run once with NTFF tracing enabled; returns max per-core exec ns."""
    elem_weights = np.asarray(np_inputs["elem_weights"], dtype=np.float32)
    elem_fea = np.asarray(np_inputs["elem_fea"], dtype=np.float32)
    params = np_inputs["params"]
    blobs, pow_g, pow_c = _prep_weights(params)
    key = (tuple(tuple(r) for r in pow_g), tuple(pow_c))
    if key not in _NC_CACHE:
        _NC_CACHE[key] = build_nc(pow_g, pow_c)
    nc = _NC_CACHE[key]
    in_maps = []
    for k in range(NCORES):
        ns = slice(N_LOC * k, N_LOC * (k + 1))
        xfea = np.zeros((256, N_LOC), np.float32)
        xfea[:200, :] = elem_fea[ns].T
        xfea[200, :] = elem_weights[ns, 0]
        m = dict(blobs)
        m["xfea"] = xfea
        m["ew"] = np.ascontiguousarray(elem_weights[ns, 0])
        in_maps.append(m)
    res = run_bass_kernel_spmd(nc, in_maps, core_ids=list(range(NCORES)),
                               trace=True)
    if res.instructions_and_trace is not None:
        print(f"trace: {res.instructions_and_trace[1]}")
    return res.exec_time_ns


# revision 32
# speedup vs baseline: 2.8237x; 2.8237x over previous
"""Trainium2 Bass kernel for nn_DescriptorNetwork (gnn_message_passing).

Data-parallel over crystals: 8192 crystals sharded 1024-per-core across 8
NeuronCores; params replicated. Each crystal has E=4 nodes, fully connected
(16 edges). All segment reductions are device-local groups of 4.

Layout: feature-major activations xT [128 part, 2 k-tiles, 4096 nodes],
fp32r matmuls (full PE rate), pair-MLP decomposed into self/nbr halves with
broadcast-pattern tensor_tensor adds, attention gates computed via a
shifted-column stationary matrix so each 256-node chunk's gate logits land
on their own PSUM partition row.
"""

import contextlib

import numpy as np

import concourse.bass as bass
import concourse.mybir as mybir
import concourse.tile as tile
from concourse import bacc
from concourse.bass_utils import run_bass_kernel_spmd

AF = mybir.ActivationFunctionType
ALU = mybir.AluOpType
X_AX = mybir.AxisListType.X
XY_AX = mybir.AxisListType.XY
F32 = mybir.dt.float32
BF16 = mybir.dt.bfloat16

# Problem constants (hardcoded per spec)
NCORES = 8
C_TOT, E = 8192, 4
N_TOT = C_TOT * E          # 32768
ELEM_EMB, FEA, HID = 200, 256, 512
N_GRAPH, HEADS = 3, 3
SLOPE = 0.01

# Per-core sizes
C_LOC = C_TOT // NCORES    # 1024 crystals
N_LOC = C_LOC * E          # 4096 nodes
M_LOC = N_LOC * E          # 16384 edges
NCH = 256                  # nodes per chunk
CHUNKS = N_LOC // NCH      # 16
ECH = NCH * E              # 1024 edges per chunk

# Shifted gate-weight buffer: w2 at col P, window c = [P-c, P-c+128)
WSH_P = CHUNKS - 1         # 15
WSH_W = WSH_P + 128        # 143

# Weight blob column offsets
WG_COLS = 2 * 512 * 2 + 4 * WSH_W            # 2620
WM_COLS = 2 * 512 * 2 + 4 * 256              # 3072
CW_COLS = 2 * 512 * 2 + 4 * WSH_W + 4 * 256  # 3644

_NC_CACHE = {}


def _edge_aps(a_base, b_base, nq):
    """Read-APs for the pair expansion over nq crystals (4*nq nodes).

    Edge order e = 16q + 4a + b (a=self offset, b=nbr offset).
    E_A[e] = A[4q+a]  -> [[1, 4nq], [0, 4]]   (each col repeated 4x)
    E_B[e] = B[4q+b]  -> [[4, nq], [0, 4], [1, 4]]
    """
    ea = bass.AP(tensor=a_base.tensor, offset=a_base.offset,
                 ap=[a_base.ap[0], [1, 4 * nq], [0, 4]])
    eb = bass.AP(tensor=b_base.tensor, offset=b_base.offset,
                 ap=[b_base.ap[0], [4, nq], [0, 4], [1, 4]])
    return ea, eb


def _bcast4(t, n):
    """[16, n] tile -> [16, n, 4] broadcast AP (each col repeated 4x)."""
    return bass.AP(tensor=t.tensor, offset=t.offset,
                   ap=[t.ap[0], [1, n], [0, 4]])


def build_nc(pow_g, pow_c):
    nc = bacc.Bacc("TRN2", target_bir_lowering=False)

    xfea = nc.dram_tensor("xfea", [256, N_LOC], BF16, kind="ExternalInput")
    ew = nc.dram_tensor("ew", [N_LOC], F32, kind="ExternalInput")
    embw = nc.dram_tensor("embw", [128, 2 * 256], BF16, kind="ExternalInput")
    hcb = nc.dram_tensor("hcb", [128, 8], F32, kind="ExternalInput")
    bcol = nc.dram_tensor("bcol", [128, 104], F32, kind="ExternalInput")
    gw = [[nc.dram_tensor(f"g{L}h{h}w", [128, WG_COLS], BF16, kind="ExternalInput")
           for h in range(HEADS)] for L in range(N_GRAPH)]
    mw = [[nc.dram_tensor(f"g{L}h{h}m", [128, WM_COLS], BF16, kind="ExternalInput")
           for h in range(HEADS)] for L in range(N_GRAPH)]
    cw = [nc.dram_tensor(f"c{h}w", [128, CW_COLS], BF16, kind="ExternalInput")
          for h in range(HEADS)]
    outT = nc.dram_tensor("outT", [256, C_LOC], F32, kind="ExternalOutput")

    with tile.TileContext(nc) as tc, contextlib.ExitStack() as ctx:
        consts = ctx.enter_context(tc.tile_pool(name="consts", bufs=1))
        gfdp = ctx.enter_context(tc.tile_pool(name="gfdp", bufs=4, space="DRAM"))
        state = ctx.enter_context(tc.tile_pool(name="state", bufs=1))

        hcb_sb = consts.tile([128, 8], F32)
        nc.sync.dma_start(out=hcb_sb, in_=hcb[:, :])
        bcol_sb = consts.tile([128, 104], F32)
        nc.sync.dma_start(out=bcol_sb, in_=bcol[:, :])
        lnw = consts.tile([16, 256], F32)
        ew_sb = consts.tile([16, 256], F32)
        nc.sync.dma_start(out=ew_sb, in_=ew.ap().rearrange("(p n) -> p n", p=16))
        nc.scalar.activation(out=lnw, in_=ew_sb, func=AF.Ln)

        xT = state.tile([128, 2, N_LOC], BF16)

        # ---------------- embed ----------------
        with tc.tile_pool(name="xfeap", bufs=1) as xfp, \
             tc.tile_pool(name="embps", bufs=2, space="PSUM") as embps:
            xf = xfp.tile([128, 2, N_LOC], BF16)
            nc.sync.dma_start(out=xf, in_=xfea.ap().rearrange("(k p) n -> p k n", p=128))
            ew_t = xfp.tile([128, 2 * 256], BF16)
            nc.sync.dma_start(out=ew_t, in_=embw[:, :])
            for c in range(8):
                cs = slice(512 * c, 512 * (c + 1))
                xp = embps.tile([128, 2, 512], F32)
                for mt in range(2):
                    for kt in range(2):
                        nc.tensor.matmul(
                            xp[:, mt, :],
                            ew_t[:, kt * 256 + mt * 128: kt * 256 + (mt + 1) * 128],
                            xf[:, kt, cs], start=(kt == 0), stop=(kt == 1))
                    nc.scalar.activation(out=xT[:, mt, cs], in_=xp[:, mt, :],
                                         func=AF.Identity,
                                         bias=bcol_sb[:, 96 + mt: 97 + mt])

        # ---------------- graph layers ----------------
        with tc.tile_pool(name="wgp", bufs=3) as wgp, \
             tc.tile_pool(name="wmp", bufs=3) as wmp, \
             tc.tile_pool(name="uhp", bufs=2) as uhp, \
             tc.tile_pool(name="drp", bufs=2) as drp, \
             tc.tile_pool(name="grp", bufs=3) as grp, \
             tc.tile_pool(name="smx", bufs=1) as smx, \
             tc.tile_pool(name="gfp", bufs=3) as gfp, \
             tc.tile_pool(name="gmp", bufs=2) as gmp, \
             tc.tile_pool(name="rdp", bufs=2) as rdp, \
             tc.tile_pool(name="lps", bufs=2, space="PSUM") as lps:
            for L in range(N_GRAPH):
                wg_t = [wgp.tile([128, WG_COLS], BF16, tag="wg", name=f"wgt{L}_{i}") for i in range(HEADS)]
                wm_t = [wmp.tile([128, WM_COLS], BF16, tag="wm", name=f"wmt{L}_{i}") for i in range(HEADS)]
                for h in range(HEADS):
                    nc.sync.dma_start(out=wg_t[h], in_=gw[L][h][:, :])
                    nc.sync.dma_start(out=wm_t[h], in_=mw[L][h][:, :])

                # ---- PASS G: gates per head (pend queues flow across heads) ----
                gfin = []
                if True:
                    gps = apsp = bpsp = lps

                    def emit_gate_mms(wgh, g_ps, c, hg):
                        for kt in range(4):
                            base = 2048 + kt * WSH_W + WSH_P - c
                            for hf in range(2):
                                nc.tensor.matmul(
                                    g_ps[:, hf * 512:(hf + 1) * 512],
                                    wgh[:, base: base + 128],
                                    hg[:, kt, hf * 512:(hf + 1) * 512],
                                    start=(c == 0 and kt == 0),
                                    stop=(c == CHUNKS - 1 and kt == 3),
                                    skip_group_check=True)

                    def emit_softmax(h, g_ps, wp):
                        gates = smx.tile([16, 1024], F32, tag="gates",
                                         name=f"gates{L}_{h}")
                        nc.vector.tensor_copy(out=gates, in_=g_ps[0:16, :])
                        gmax = smx.tile([16, 256], F32, tag="gmax",
                                        name=f"gmax{L}_{h}")
                        nc.vector.tensor_reduce(
                            out=gmax, in_=gates.rearrange("p (g j) -> p g j", j=4),
                            axis=X_AX, op=ALU.max)
                        nc.vector.tensor_tensor(out=gates, in0=gates,
                                                in1=_bcast4(gmax, 256),
                                                op=ALU.subtract)
                        nc.scalar.activation(out=gates, in_=gates, func=AF.Exp)
                        wpe = bass.AP(tensor=wp.tensor, offset=wp.offset,
                                      ap=[wp.ap[0], [4, 64], [0, 4], [1, 4]])
                        gv = gates.rearrange("p (a b c) -> p a b c", b=4, c=4)
                        nc.vector.tensor_tensor(out=gv, in0=wpe, in1=gv, op=ALU.mult)
                        ssum = smx.tile([16, 256], F32, tag="ssum",
                                        name=f"ssum{L}_{h}")
                        nc.vector.tensor_reduce(
                            out=ssum, in_=gates.rearrange("p (g j) -> p g j", j=4),
                            axis=X_AX, op=ALU.add)
                        nc.vector.tensor_scalar_add(out=ssum, in0=ssum, scalar1=1e-10)
                        rinv = smx.tile([16, 256], F32, tag="rinv",
                                        name=f"rinv{L}_{h}")
                        nc.vector.reciprocal(out=rinv, in_=ssum)
                        gf = gfp.tile([16, 1024], F32, tag="gf", name=f"gf{L}_{h}")
                        nc.vector.tensor_tensor(out=gf, in0=gates,
                                                in1=_bcast4(rinv, 256), op=ALU.mult)
                        gfd = gfdp.tile([16, 1024], F32, tag="gfd",
                                        name=f"gfd{L}_{h}")
                        nc.sync.dma_start(out=gfd, in_=gf)
                        gfin.append(gfd)

                    pend_lr = []
                    pend_mm = []
                    for h in range(HEADS):
                        wgh = wg_t[h]
                        wp = smx.tile([16, 256], F32, tag="wp", name=f"wp{L}_{h}", bufs=2)
                        nc.scalar.activation(out=wp, in_=lnw, func=AF.Exp,
                                             scale=float(pow_g[L][h]))
                        bG = (L * 3 + h) * 8
                        g_ps = gps.tile([128, 1024], F32, tag="big",
                                        name=f"gps{L}_{h}")
                        for c in range(CHUNKS // 2):
                            cs = slice(512 * c, 512 * (c + 1))
                            asb = drp.tile([128, 4, 512], BF16, tag="dr0")
                            bsb = drp.tile([128, 4, 512], BF16, tag="dr1")
                            for mt in range(4):
                                amt = apsp.tile([128, 512], F32, tag="aps",
                                                name=f"ga{L}_{h}_{c}_{mt}")
                                for kt in range(2):
                                    nc.tensor.matmul(
                                        amt,
                                        wgh[:, kt * 512 + mt * 128: kt * 512 + (mt + 1) * 128],
                                        xT[:, kt, cs], start=(kt == 0), stop=(kt == 1))
                                nc.vector.tensor_copy(out=asb[:, mt, :], in_=amt)
                                bmt = bpsp.tile([128, 512], F32, tag="bps",
                                                name=f"gb{L}_{h}_{c}_{mt}")
                                for kt in range(2):
                                    nc.tensor.matmul(
                                        bmt,
                                        wgh[:, 1024 + kt * 512 + mt * 128:
                                            1024 + kt * 512 + (mt + 1) * 128],
                                        xT[:, kt, cs], start=(kt == 0), stop=(kt == 1))
                                nc.scalar.activation(
                                    out=bsb[:, mt, :], in_=bmt, func=AF.Identity,
                                    bias=bcol_sb[:, bG + mt: bG + mt + 1])
                            for half in range(2):
                                r = 2 * c + half
                                hs = slice(256 * half, 256 * (half + 1))
                                ug = uhp.tile([128, 4, 1024], BF16, tag="u", bufs=3)
                                for mt in range(4):
                                    ea, ebp = _edge_aps(asb[:, mt, hs],
                                                        bsb[:, mt, hs], 64)
                                    eng = nc.gpsimd if mt < 2 else nc.vector
                                    eng.tensor_tensor(out=ug[:, mt, :], in0=ebp,
                                                      in1=ea, op=ALU.add)
                                pend_lr.append((wgh, g_ps, h, wp, r, ug))
                                if len(pend_lr) > 1:
                                    pw, pg, ph, pwp, pr, pug = pend_lr.pop(0)
                                    hg = uhp.tile([128, 4, 1024], BF16, tag="h",
                                                  bufs=4, name=f"hg{L}_{ph}_{pr}")
                                    nc.scalar.activation(out=hg, in_=pug,
                                                         func=AF.Lrelu, alpha=SLOPE)
                                    pend_mm.append((pw, pg, ph, pwp, pr, hg))
                                if len(pend_mm) > 1:
                                    pw, pg, ph, pwp, pr, phg = pend_mm.pop(0)
                                    emit_gate_mms(pw, pg, pr, phg)
                                    if pr == CHUNKS - 1:
                                        emit_softmax(ph, pg, pwp)
                    for pw, pg, ph, pwp, pr, pug in pend_lr:
                        hg = uhp.tile([128, 4, 1024], BF16, tag="h", bufs=4,
                                      name=f"hgf{L}_{ph}_{pr}")
                        nc.scalar.activation(out=hg, in_=pug, func=AF.Lrelu,
                                             alpha=SLOPE)
                        pend_mm.append((pw, pg, ph, pwp, pr, hg))
                    for pw, pg, ph, pwp, pr, phg in pend_mm:
                        emit_gate_mms(pw, pg, pr, phg)
                        if pr == CHUNKS - 1:
                            emit_softmax(ph, pg, pwp)

                # ---- PASS M: messages, chunks outer / heads inner ----
                if True:
                    mps = apsp = bpsp = lps
                    def emit_msgout(c, h, hm, grep, gmsg):
                        wmh = wm_t[h]
                        for ft in range(2):
                            mp = mps.tile([128, 1024], F32, tag="big",
                                          name=f"mp{L}_{c}_{h}_{ft}")
                            for kt in range(4):
                                for hf in range(2):
                                    nc.tensor.matmul(
                                        mp[:, hf * 512:(hf + 1) * 512],
                                        wmh[:, 2048 + kt * 256 + ft * 128:
                                            2048 + kt * 256 + (ft + 1) * 128],
                                        hm[:, kt, hf * 512:(hf + 1) * 512],
                                        start=(kt == 0), stop=(kt == 3),
                                        skip_group_check=True)
                            nc.vector.tensor_tensor(out=gmsg[:, ft, h, :],
                                                    in0=mp, in1=grep,
                                                    op=ALU.mult)

                    def emit_reduce(c, gmsg):
                        cs = slice(NCH * c, NCH * (c + 1))
                        for ft in range(2):
                            red = rdp.tile([128, 256], F32, tag="red",
                                           name=f"red{L}_{c}_{ft}")
                            b0 = gmsg[:, ft, 0, :]
                            rin = bass.AP(tensor=b0.tensor, offset=b0.offset,
                                          ap=[b0.ap[0], [4, 256], [1024, 3], [1, 4]])
                            nc.vector.tensor_reduce(out=red, in_=rin, axis=XY_AX,
                                                    op=ALU.add)
                            nc.vector.tensor_scalar_add(
                                out=red, in0=red,
                                scalar1=hcb_sb[:, 2 * L + ft: 2 * L + ft + 1])
                            nc.gpsimd.tensor_tensor(out=xT[:, ft, cs], in0=red,
                                                    in1=xT[:, ft, cs], op=ALU.add)

                    pend_lr = []
                    pend_mm = []
                    for c in range(CHUNKS // 2):
                        cs = slice(512 * c, 512 * (c + 1))
                        gms = {}
                        for half in range(2):
                            r = 2 * c + half
                            gms[half] = gmp.tile([128, 2, 3, 1024], BF16,
                                                 tag="gmsg", bufs=3,
                                                 name=f"gmsg{L}_{r}")
                        for h in range(HEADS):
                            wmh = wm_t[h]
                            bM = (L * 3 + h) * 8 + 4
                            asb = drp.tile([128, 4, 512], BF16, tag="dr1")
                            bsb = drp.tile([128, 4, 512], BF16, tag="dr2")
                            for mt in range(4):
                                amt = apsp.tile([128, 512], F32, tag="aps",
                                                name=f"ma{L}_{c}_{h}_{mt}")
                                for kt in range(2):
                                    nc.tensor.matmul(
                                        amt,
                                        wmh[:, kt * 512 + mt * 128: kt * 512 + (mt + 1) * 128],
                                        xT[:, kt, cs], start=(kt == 0), stop=(kt == 1))
                                nc.scalar.activation(
                                    out=asb[:, mt, :], in_=amt, func=AF.Identity,
                                    bias=bcol_sb[:, bM + mt: bM + mt + 1])
                                bmt = bpsp.tile([128, 512], F32, tag="bps",
                                                name=f"mb{L}_{c}_{h}_{mt}")
                                for kt in range(2):
                                    nc.tensor.matmul(
                                        bmt,
                                        wmh[:, 1024 + kt * 512 + mt * 128:
                                            1024 + kt * 512 + (mt + 1) * 128],
                                        xT[:, kt, cs], start=(kt == 0), stop=(kt == 1))
                                nc.vector.tensor_copy(out=bsb[:, mt, :], in_=bmt)
                            for half in range(2):
                                r = 2 * c + half
                                hs = slice(256 * half, 256 * (half + 1))
                                um = uhp.tile([128, 4, 1024], BF16, tag="u", bufs=3)
                                for mt in range(4):
                                    ea, ebp = _edge_aps(asb[:, mt, hs],
                                                        bsb[:, mt, hs], 64)
                                    eng = nc.vector if mt == 3 else nc.gpsimd
                                    eng.tensor_tensor(out=um[:, mt, :], in0=ebp,
                                                      in1=ea, op=ALU.add)
                                grep = grp.tile([128, 1024], F32, tag="grep",
                                                bufs=4)
                                gsrc = gfin[h][r:r + 1, :]
                                bc = bass.AP(tensor=gsrc.tensor, offset=gsrc.offset,
                                             ap=[[0, 128], [1, 1024]])
                                nc.sync.dma_start(out=grep, in_=bc)
                                pend_lr.append((r, h, um, grep, gms[half]))
                                if len(pend_lr) > 1:
                                    pr, ph, pum, pgrep, pgm = pend_lr.pop(0)
                                    hm = uhp.tile([128, 4, 1024], BF16, tag="h",
                                                  bufs=4, name=f"hm{L}_{pr}_{ph}")
                                    nc.scalar.activation(out=hm, in_=pum,
                                                         func=AF.Lrelu, alpha=SLOPE)
                                    pend_mm.append((pr, ph, hm, pgrep, pgm))
                                if len(pend_mm) > 1:
                                    pr, ph, phm, pgrep, pgm = pend_mm.pop(0)
                                    emit_msgout(pr, ph, phm, pgrep, pgm)
                                    if ph == HEADS - 1:
                                        emit_reduce(pr, pgm)
                    for pr, ph, pum, pgrep, pgm in pend_lr:
                        hm = uhp.tile([128, 4, 1024], BF16, tag="h", bufs=4,
                                      name=f"hmf{L}_{pr}_{ph}")
                        nc.scalar.activation(out=hm, in_=pum, func=AF.Lrelu,
                                             alpha=SLOPE)
                        pend_mm.append((pr, ph, hm, pgrep, pgm))
                    for pr, ph, phm, pgrep, pgm in pend_mm:
                        emit_msgout(pr, ph, phm, pgrep, pgm)
                        if ph == HEADS - 1:
                            emit_reduce(pr, pgm)

        # ---------------- crystal pooling ----------------
        with tc.tile_pool(name="cwp", bufs=3) as cwp, \
             tc.tile_pool(name="cstate", bufs=1) as cstate, \
             tc.tile_pool(name="cuh", bufs=2) as cuh, \
             tc.tile_pool(name="cgr", bufs=1) as cgr, \
             tc.tile_pool(name="csx", bufs=1) as csx, \
             tc.tile_pool(name="cgf", bufs=3) as cgfp, \
             tc.tile_pool(name="cgm", bufs=1) as cgm:
            ocry = cstate.tile([128, 2, C_LOC], F32)
            cw_t = [cwp.tile([128, CW_COLS], BF16, tag="cw", name=f"cwt{i}") for i in range(HEADS)]
            for h in range(HEADS):
                nc.sync.dma_start(out=cw_t[h], in_=cw[h][:, :])
            cfin = []
            for h in range(HEADS):
                wch = cw_t[h]
                bC = 72 + h * 8
                wp = csx.tile([16, 256], F32, tag="cwpow")
                nc.scalar.activation(out=wp, in_=lnw, func=AF.Exp,
                                     scale=float(pow_c[h]))
                with tc.tile_pool(name="cgpsG", bufs=1, space="PSUM") as gps, \
                     tc.tile_pool(name="cabG", bufs=2, space="PSUM") as abps:
                    g_ps = gps.tile([128, 256], F32, tag="cgps")

                    def emit_cgate(c, hcr):
                        for kt in range(4):
                            base = 2048 + kt * WSH_W + WSH_P - c
                            nc.tensor.matmul(
                                g_ps[:, :], wch[:, base: base + 128], hcr[:, kt, :],
                                start=(c == 0 and kt == 0),
                                stop=(c == CHUNKS - 1 and kt == 3),
                                skip_group_check=True)

                    pend = []
                    for c in range(CHUNKS):
                        cs = slice(NCH * c, NCH * (c + 1))
                        ap_ = abps.tile([128, 4, 256], F32, tag="cab")
                        for mt in range(4):
                            for kt in range(2):
                                nc.tensor.matmul(
                                    ap_[:, mt, :],
                                    wch[:, kt * 512 + mt * 128: kt * 512 + (mt + 1) * 128],
                                    xT[:, kt, cs], start=(kt == 0), stop=(kt == 1))
                        hcr = cuh.tile([128, 4, 256], BF16, tag="cuh", bufs=4)
                        for mt in range(4):
                            nc.scalar.activation(out=hcr[:, mt, :], in_=ap_[:, mt, :],
                                                 func=AF.Lrelu, alpha=SLOPE,
                                                 bias=bcol_sb[:, bC + mt: bC + mt + 1])
                        pend.append((c, hcr))
                        if len(pend) > 2:
                            emit_cgate(*pend.pop(0))
                    for p in pend:
                        emit_cgate(*p)
                    gates = csx.tile([16, 256], F32, tag="cgates")
                    nc.vector.tensor_copy(out=gates, in_=g_ps[0:16, :])
                gmax = csx.tile([16, 64], F32, tag="cgmax")
                nc.vector.tensor_reduce(
                    out=gmax, in_=gates.rearrange("p (g j) -> p g j", j=4),
                    axis=X_AX, op=ALU.max)
                nc.vector.tensor_tensor(out=gates, in0=gates,
                                        in1=_bcast4(gmax, 64), op=ALU.subtract)
                nc.scalar.activation(out=gates, in_=gates, func=AF.Exp)
                nc.vector.tensor_tensor(out=gates, in0=gates, in1=wp, op=ALU.mult)
                ssum = csx.tile([16, 64], F32, tag="cssum")
                nc.vector.tensor_reduce(
                    out=ssum, in_=gates.rearrange("p (g j) -> p g j", j=4),
                    axis=X_AX, op=ALU.add)
                nc.vector.tensor_scalar_add(out=ssum, in0=ssum, scalar1=1e-10)
                rinv = csx.tile([16, 64], F32, tag="crinv")
                nc.vector.reciprocal(out=rinv, in_=ssum)
                gf = cgfp.tile([16, 256], F32, tag="cgf")
                nc.vector.tensor_tensor(out=gf, in0=gates, in1=_bcast4(rinv, 64),
                                        op=ALU.mult)
                gfd = gfdp.tile([16, 256], F32, tag="cgfd", name=f"cgfd{h}")
                nc.sync.dma_start(out=gfd, in_=gf)
                cfin.append(gfd)

            with tc.tile_pool(name="cmpsM", bufs=2, space="PSUM") as mps, \
                 tc.tile_pool(name="cabM", bufs=2, space="PSUM") as abps:
                def emit_cmsg(c, h, hcr, grep, gmsg):
                    wch = cw_t[h]
                    mp = mps.tile([128, 2, 256], F32, tag="cmp",
                                  name=f"cmp{c}_{h}")
                    for ft in range(2):
                        for kt in range(4):
                            nc.tensor.matmul(
                                mp[:, ft, :],
                                wch[:, 2620 + kt * 256 + ft * 128:
                                    2620 + kt * 256 + (ft + 1) * 128],
                                hcr[:, kt, :], start=(kt == 0), stop=(kt == 3),
                                skip_group_check=True)
                    for ft in range(2):
                        nc.vector.tensor_tensor(out=gmsg[:, ft, h, :],
                                                in0=mp[:, ft, :], in1=grep,
                                                op=ALU.mult)

                def emit_cred(c, gmsg):
                    for ft in range(2):
                        b0 = gmsg[:, ft, 0, :]
                        rin = bass.AP(tensor=b0.tensor, offset=b0.offset,
                                      ap=[b0.ap[0], [4, 64], [256, 3], [1, 4]])
                        nc.vector.tensor_reduce(out=ocry[:, ft, 64 * c: 64 * (c + 1)],
                                                in_=rin, axis=XY_AX, op=ALU.add)

                pend = []
                for c in range(CHUNKS):
                    cs = slice(NCH * c, NCH * (c + 1))
                    gmsg = cgm.tile([128, 2, 3, 256], F32, tag="cgmsg", bufs=2,
                                    name=f"cgmsg{c}")
                    for h in range(HEADS):
                        wch = cw_t[h]
                        bC = 72 + h * 8 + 4
                        ap_ = abps.tile([128, 4, 256], F32, tag="cab")
                        for mt in range(4):
                            for kt in range(2):
                                nc.tensor.matmul(
                                    ap_[:, mt, :],
                                    wch[:, 1024 + kt * 512 + mt * 128:
                                        1024 + kt * 512 + (mt + 1) * 128],
                                    xT[:, kt, cs], start=(kt == 0), stop=(kt == 1))
                        hcr = cuh.tile([128, 4, 256], BF16, tag="cuh", bufs=4)
                        for mt in range(4):
                            nc.scalar.activation(out=hcr[:, mt, :], in_=ap_[:, mt, :],
                                                 func=AF.Lrelu, alpha=SLOPE,
                                                 bias=bcol_sb[:, bC + mt: bC + mt + 1])
                        grep = cgr.tile([128, 256], F32, tag="cgrep", bufs=4)
                        gsrc = cfin[h][c:c + 1, :]
                        bc = bass.AP(tensor=gsrc.tensor, offset=gsrc.offset,
                                     ap=[[0, 128], [1, 256]])
                        nc.sync.dma_start(out=grep, in_=bc)
                        pend.append((c, h, hcr, grep, gmsg))
                        if len(pend) > 2:
                            pc, ph, phcr, pgrep, pgm = pend.pop(0)
                            emit_cmsg(pc, ph, phcr, pgrep, pgm)
                            if ph == HEADS - 1:
                                emit_cred(pc, pgm)
                for pc, ph, phcr, pgrep, pgm in pend:
                    emit_cmsg(pc, ph, phcr, pgrep, pgm)
                    if ph == HEADS - 1:
                        emit_cred(pc, pgm)
            for ft in range(2):
                nc.vector.tensor_scalar_add(out=ocry[:, ft, :], in0=ocry[:, ft, :],
                                            scalar1=hcb_sb[:, 6 + ft: 7 + ft])
            nc.sync.dma_start(out=outT.ap().rearrange("(k p) n -> p k n", p=128),
                              in_=ocry)

    nc.finalize()
    return nc


def _prep_weights(params):
    """Pack parameter pytree into the DRAM blob arrays (host-side, f32)."""
    g = lambda x: np.asarray(x, dtype=np.float32)
    blobs = {}

    def pack_fc1(W):
        din = W.shape[0]
        nk = din // 128
        out = np.zeros((128, nk * 512), np.float32)
        for kt in range(nk):
            out[:, kt * 512:(kt + 1) * 512] = W[kt * 128:(kt + 1) * 128, :]
        return out

    def pack_w2sh(w2):
        out = np.zeros((128, 4 * WSH_W), np.float32)
        for kt in range(4):
            out[:, kt * WSH_W + WSH_P] = w2[kt * 128:(kt + 1) * 128, 0]
        return out

    def pack_w2m(W2):
        out = np.zeros((128, 4 * 256), np.float32)
        for kt in range(4):
            out[:, kt * 256:(kt + 1) * 256] = W2[kt * 128:(kt + 1) * 128, :] / 3.0
        return out

    bcol_arr = np.zeros((128, 104), np.float32)
    pow_g = [[0.0] * HEADS for _ in range(N_GRAPH)]
    pow_c = [0.0] * HEADS
    for L in range(N_GRAPH):
        for h in range(HEADS):
            p = params["graphs"][L][h]
            Wg, Wm = g(p["gate"]["fc1"]["W"]), g(p["msg"]["fc1"]["W"])
            blobs[f"g{L}h{h}w"] = np.concatenate(
                [pack_fc1(Wg[:256]), pack_fc1(Wg[256:]),
                 pack_w2sh(g(p["gate"]["out"]["W"]))], axis=1)
            blobs[f"g{L}h{h}m"] = np.concatenate(
                [pack_fc1(Wm[:256]), pack_fc1(Wm[256:]),
                 pack_w2m(g(p["msg"]["out"]["W"]))], axis=1)
            bcol_arr[:, (L * 3 + h) * 8: (L * 3 + h) * 8 + 4] = \
                g(p["gate"]["fc1"]["b"]).reshape(4, 128).T
            bcol_arr[:, (L * 3 + h) * 8 + 4: (L * 3 + h) * 8 + 8] = \
                g(p["msg"]["fc1"]["b"]).reshape(4, 128).T
            pow_g[L][h] = float(np.asarray(p["pow"]).reshape(-1)[0])
    for h in range(HEADS):
        p = params["cry_pool"][h]
        blobs[f"c{h}w"] = np.concatenate(
            [pack_fc1(g(p["gate"]["fc1"]["W"])), pack_fc1(g(p["msg"]["fc1"]["W"])),
             pack_w2sh(g(p["gate"]["out"]["W"])),
             pack_w2m(g(p["msg"]["out"]["W"]))], axis=1)
        bcol_arr[:, 72 + h * 8: 72 + h * 8 + 4] = \
            g(p["gate"]["fc1"]["b"]).reshape(4, 128).T
        bcol_arr[:, 72 + h * 8 + 4: 72 + h * 8 + 8] = \
            g(p["msg"]["fc1"]["b"]).reshape(4, 128).T
        pow_c[h] = float(np.asarray(p["pow"]).reshape(-1)[0])

    embW = g(params["embed"]["W"])
    Wt = np.zeros((256, 256), np.float32)
    Wt[:200, :255] = embW
    Wt[200, 255] = 1.0
    embarr = np.zeros((128, 512), np.float32)
    for kt in range(2):
        embarr[:, kt * 256:(kt + 1) * 256] = Wt[kt * 128:(kt + 1) * 128, :]
    blobs["embw"] = embarr
    ebf = np.zeros(256, np.float32)
    ebf[:255] = g(params["embed"]["b"])
    bcol_arr[:, 96] = ebf[:128]
    bcol_arr[:, 97] = ebf[128:]

    hcbarr = np.zeros((128, 8), np.float32)
    for L in range(N_GRAPH):
        b = sum(g(params["graphs"][L][h]["msg"]["out"]["b"]) for h in range(HEADS)) / 3.0
        hcbarr[:, 2 * L] = b[:128]
        hcbarr[:, 2 * L + 1] = b[128:]
    bc = sum(g(params["cry_pool"][h]["msg"]["out"]["b"]) for h in range(HEADS)) / 3.0
    hcbarr[:, 6] = bc[:128]
    hcbarr[:, 7] = bc[128:]
    blobs["hcb"] = hcbarr
    blobs["ones"] = np.ones((1, 512), np.float32)
    return blobs, pow_g, pow_c


def _check_indices(self_idx, nbr_idx, cry_idx):
    node = np.arange(N_TOT, dtype=np.int64).reshape(C_TOT, E)
    exp_self = np.repeat(node, E, axis=1).reshape(-1)
    exp_nbr = np.tile(node, (1, E)).reshape(-1)
    exp_cry = np.repeat(np.arange(C_TOT, dtype=np.int64), E)
    return (np.array_equal(np.asarray(self_idx, np.int64), exp_self)
            and np.array_equal(np.asarray(nbr_idx, np.int64), exp_nbr)
            and np.array_equal(np.asarray(cry_idx, np.int64), exp_cry))


def _numpy_reference(elem_weights, elem_fea, self_idx, nbr_idx, cry_idx, params):
    """Pure-numpy fallback used only if the index structure ever differs."""
    g = lambda x: np.asarray(x, dtype=np.float32)

    def net(p, x):
        h = x @ g(p["fc1"]["W"]) + g(p["fc1"]["b"])
        h = np.where(h > 0, h, SLOPE * h)
        return h @ g(p["out"]["W"]) + g(p["out"]["b"])

    def seg_max(x, idx, n):
        out = np.full((n,) + x.shape[1:], -np.inf, np.float32)
        np.maximum.at(out, idx, x)
        return out

    def seg_sum(x, idx, n):
        out = np.zeros((n,) + x.shape[1:], np.float32)
        np.add.at(out, idx, x)
        return out

    def attn_pool(p, x, idx, w, n):
        gate = net(p["gate"], x)
        gate = gate - seg_max(gate, idx, n)[idx]
        gate = (w ** float(np.asarray(p["pow"]).reshape(-1)[0])) * np.exp(gate)
        gate = gate / (seg_sum(gate, idx, n)[idx] + 1e-10)
        return seg_sum(gate * net(p["msg"], x), idx, n)

    ew = g(elem_weights)
    x = g(elem_fea) @ g(params["embed"]["W"]) + g(params["embed"]["b"])
    x = np.concatenate([x, ew], axis=1)
    for heads in params["graphs"]:
        pair = np.concatenate([x[self_idx], x[nbr_idx]], axis=1)
        nw = ew[nbr_idx]
        msg = np.mean([attn_pool(p, pair, self_idx, nw, N_TOT) for p in heads], axis=0)
        x = msg + x
    return np.mean([attn_pool(p, x, cry_idx, ew, C_TOT) for p in params["cry_pool"]],
                   axis=0)


def kernel(elem_weights, elem_fea, self_fea_idx, nbr_fea_idx, cry_elem_idx, params):
    elem_weights = np.asarray(elem_weights, dtype=np.float32)
    elem_fea = np.asarray(elem_fea, dtype=np.float32)
    if not _check_indices(self_fea_idx, nbr_fea_idx, cry_elem_idx):
        return _numpy_reference(elem_weights, elem_fea,
                                np.asarray(self_fea_idx, np.int64),
                                np.asarray(nbr_fea_idx, np.int64),
                                np.asarray(cry_elem_idx, np.int64), params)

    blobs, pow_g, pow_c = _prep_weights(params)
    key = (tuple(tuple(r) for r in pow_g), tuple(pow_c))
    if key not in _NC_CACHE:
        _NC_CACHE[key] = build_nc(pow_g, pow_c)
    nc = _NC_CACHE[key]

    in_maps = []
    for k in range(NCORES):
        ns = slice(N_LOC * k, N_LOC * (k + 1))
        xfea = np.zeros((256, N_LOC), np.float32)
        xfea[:200, :] = elem_fea[ns].T
        xfea[200, :] = elem_weights[ns, 0]
        m = dict(blobs)
        m["xfea"] = xfea
        m["ew"] = np.ascontiguousarray(elem_weights[ns, 0])
        in_maps.append(m)

    res = run_bass_kernel_spmd(nc, in_maps, core_ids=list(range(NCORES)))
    out = np.empty((C_TOT, FEA), np.float32)
    for k in range(NCORES):
        out[C_LOC * k: C_LOC * (k + 1), :] = res.results[k]["outT"].T
    return out


def timed_run(np_inputs):
    """Run once with NTFF tracing enabled; returns max per-core exec ns."""
    elem_weights = np.asarray(np_inputs["elem_weights"], dtype=np.float32)
    elem_fea = np.asarray(np_inputs["elem_fea"], dtype=np.float32)
    params = np_inputs["params"]
    blobs, pow_g, pow_c = _prep_weights(params)
    key = (tuple(tuple(r) for r in pow_g), tuple(pow_c))
    if key not in _NC_CACHE:
        _NC_CACHE[key] = build_nc(pow_g, pow_c)
    nc = _NC_CACHE[key]
    in_maps = []
    for k in range(NCORES):
        ns = slice(N_LOC * k, N_LOC * (k + 1))
        xfea = np.zeros((256, N_LOC), np.float32)
        xfea[:200, :] = elem_fea[ns].T
        xfea[200, :] = elem_weights[ns, 0]
        m = dict(blobs)
        m["xfea"] = xfea
        m["ew"] = np.ascontiguousarray(elem_weights[ns, 0])
        in_maps.append(m)
    res = run_bass_kernel_spmd(nc, in_maps, core_ids=list(range(NCORES)),
                               trace=True)
    if res.instructions_and_trace is not None:
        print(f"trace: {res.instructions_and_trace[1]}")
    return res.exec_time_ns
